# revision 9
# baseline (speedup 1.0000x reference)
"""Dilated segment attention on 8 TRN2 NeuronCores (Bass/Tile).

Problem (hardcoded from spec):
  x [2, 8192, 2048] f32, Wqkv [6144, 2048], b_qkv [6144], Wout [2048, 2048],
  b_out [2048].  segment=512, dilation=2 -> 16 segments of L=256 dilated
  tokens per batch; per-segment 16-head attention (hd=128); fused qkv and
  out projections.  Output [2, 4096, 2048] f32.

Sharding: the 32 (batch, segment) instances are independent -> 4 per core.
Host pre-gathers the dilated tokens, pre-transposes/pre-tiles operands and
casts to bf16 (compute precision; measured end-to-end rel err ~5e-3).

Per-core dataflow (all matmuls K=128, bf16):
  qkv proj   : feature-major  qkvT[e, tok] = W-tile.T @ xsT-tile  (accum 16
               d-tiles); psum->sbuf drain + b_qkv on DVE (ScalarE is kept
               free for the softmax exps)
  scores     : scoresT[lk, lq] = kT.T @ qT        (per seg, head)
  softmax    : exp on ScalarE (scale=1/sqrt(hd); scores are provably in
               [-6, 6] so no max subtraction), pipelined two projection
               chunks ahead of the sums/AV consumers; sums via ones-matmul,
               normalize on DVE
  AV         : outT[hd, lq] = v[lk, hd].T @ expT[lk, lq]  (into the scores
               psum bank, post-exp)
  out proj   : out[l, e] = aT-tile.T @ WoutT-tile  (accum 16 head-tiles,
               token-major, so the HBM store is linear)
b_out is applied on the host (purely linear post-op); b_qkv is applied
on-chip (DVE tensor_scalar bias) since it feeds the softmax nonlinearity.
"""

import numpy as np
import ml_dtypes

B = 2
S = 8192
D = 2048
H = 16
HD = 128
SEGMENT = 512
DIL = 2
NSEG = S // SEGMENT          # 16
L = SEGMENT // DIL           # 256 dilated tokens per segment
N_CORES = 8
PAIRS = B * NSEG             # 32 independent (b, n) instances
SPC = PAIRS // N_CORES       # 4 segments per core
TOK = SPC * L                # 1024 tokens per core
DT = D // 128                # 16 contraction tiles
NCHUNK = 3 * D // 128        # 48 qkv feature chunks (16 q, 16 k, 16 v)
SCALE = 1.0 / float(np.sqrt(HD))

_PROGRAM = None


def _build_program():
    import concourse.bass as bass
    import concourse.bacc as bacc
    import concourse.tile as tile
    from concourse import mybir

    BF = mybir.dt.bfloat16
    F32 = mybir.dt.float32
    ts = bass.ts

    nc = bacc.Bacc("TRN2", target_bir_lowering=False, debug=False,
                   num_devices=N_CORES)

    xst_d = nc.dram_tensor("xst", [128, DT * TOK], BF, kind="ExternalInput")
    wqkv_d = nc.dram_tensor("wqkv_t", [NCHUNK, 128, DT * 128], BF,
                            kind="ExternalInput")
    wout_d = nc.dram_tensor("wout_t", [4, 128, DT * 512], BF, kind="ExternalInput")
    bq_d = nc.dram_tensor("bq_t", [128, NCHUNK], F32, kind="ExternalInput")
    out_d = nc.dram_tensor("out", [TOK, D], F32, kind="ExternalOutput")

    with tile.TileContext(nc) as tc:
        with (
            tc.tile_pool(name="const", bufs=1) as const_p,
            tc.tile_pool(name="big", bufs=1) as big_p,
            tc.tile_pool(name="wq", bufs=6) as w_p,
            tc.tile_pool(name="qk", bufs=4) as qk_p,
            tc.tile_pool(name="vt", bufs=2) as vt_p,
            tc.tile_pool(name="ex", bufs=4) as ex_p,
            tc.tile_pool(name="st", bufs=2) as st_p,
            tc.tile_pool(name="ou", bufs=2) as ou_p,
            tc.tile_pool(name="pp", bufs=4, space="PSUM") as pp_p,
            tc.tile_pool(name="pa", bufs=2, space="PSUM") as pa_p,
        ):
            bq_sb = const_p.tile([128, NCHUNK], F32)
            nc.sync.dma_start(out=bq_sb[:], in_=bq_d[:])
            ones = const_p.tile([128, 1], BF)
            nc.gpsimd.memset(ones[:], 1.0)

            # One linear 512KB DMA per weight chunk; xst arrives in 8 x 512KB
            # slices with the first two W chunks' DMAs interleaved between
            # them so chunk n+1's weights never land behind the full 4MB xst
            # stream (measured ~13us of head PE idle with 4 x 1MB up front).
            first_w = w_p.tile([128, DT * 128], BF, tag="w", name="first_w")
            nc.sync.dma_start(out=first_w[:], in_=wqkv_d[32])
            w33 = w_p.tile([128, DT * 128], BF, tag="w", name="w33")
            w34 = w_p.tile([128, DT * 128], BF, tag="w", name="w34")
            xst_sb = big_p.tile([128, DT, TOK], BF)
            for kk in range(8):
                nc.sync.dma_start(
                    out=xst_sb[:, 2 * kk:2 * (kk + 1), :],
                    in_=xst_d[:, 2 * kk * TOK:2 * (kk + 1) * TOK],
                )
                if kk == 1:
                    nc.sync.dma_start(out=w33[:], in_=wqkv_d[33])
                if kk == 3:
                    nc.sync.dma_start(out=w34[:], in_=wqkv_d[34])
            vtok_sb = big_p.tile([128, H, SPC * 2, 128], BF)
            aT_sb = big_p.tile([128, SPC, H, L], BF)

            def proj_chunk(c, out_tile, wck=None):
                """qkvT chunk c: out_tile[128, TOK] bf16 = (Wqkv chunk).T @ xsT + b.

                The psum->sbuf drain (+bias) runs on the DVE, not ScalarE:
                ScalarE is reserved for the softmax exps so they never queue
                behind a 2x700ns drain pair (measured ~165-240ns of PE stall
                per segment waiting on e_t with the drains on ScalarE).
                """
                if wck is None:
                    wck = w_p.tile([128, DT * 128], BF, tag="w")
                    nc.sync.dma_start(out=wck[:], in_=wqkv_d[c])
                pss = [pp_p.tile([128, 512], F32, tag="pp", name=f"ps{half}")
                       for half in range(2)]
                for dt in range(DT):
                    for half in range(2):
                        nc.tensor.matmul(
                            pss[half][:],
                            wck[:, ts(dt, 128)],
                            xst_sb[:, dt, ts(half, 512)],
                            start=(dt == 0),
                            stop=(dt == DT - 1),
                        )
                for half in range(2):
                    nc.vector.tensor_scalar_add(
                        out_tile[:, ts(half, 512)],
                        pss[half][:],
                        bq_sb[:, c:c + 1],
                    )

            # ---- v projection (feature-major) + transpose to token-major ----
            # One transposing DMA per head (xbar transpose, ~261GB/s): row
            # tok = tc*128+p of vt.T lands at vtok[p, tc, :], exactly the AV
            # stationary layout.  Keeps ~28us of transposes off the PE.
            # Emitted one chunk behind the projection so the DMA never waits
            # on the ScalarE psum->sbuf drain in the static stream.
            def v_transposes(h, vt_tile):
                nc.sync.dma_start(out=vtok_sb[:, h, :, :], in_=vt_tile[:],
                                  transpose=True)

            prev_v = None
            prefetched = {0: first_w, 1: w33, 2: w34}
            for h in range(H):
                vt_tile = vt_p.tile([128, TOK], BF, tag="vt")
                proj_chunk(32 + h, vt_tile, wck=prefetched.get(h))
                if prev_v is not None:
                    v_transposes(h - 1, prev_v)
                prev_v = vt_tile
            v_transposes(H - 1, prev_v)

            # ---- per-head: q/k projection then attention over 4 segments ----
            # scoresT[lk, lq] directly (operands swapped): exp is
            # layout-agnostic (scores provably small -> no max pass),
            # softmax sums go along partitions via a ones-matmul, expT
            # feeds AV untransposed, and the normalization happens at the
            # psum->sbuf copy of the AV output.
            #
            # Two-stage software pipeline against the projections:
            #   iter h: projq(h), projk(h),
            #           sums/AV/normalize(h-1)   [frees scT(h-1) banks]
            #           scores(h) + exps(h)      [exps run on an otherwise
            #                                     idle ScalarE under head
            #                                     h+1's ~13.6us of proj MMs]
            # so the sums/AV never wait on exp latency (measured ~360ns/seg
            # when exp was consumed within the same attention block).  The
            # sums and AV reuse the scores psum bank (sums -> [0:1,1,:],
            # AV -> [:,0,:], both emitted after the exp consumed the
            # scores), so attention fits in 4 banks alongside the 4
            # projection banks.  scores(h) sits ~1.7us of sums/AV work
            # after projk(h)'s last matmul, covering the DVE drain latency
            # of kh.

            sc_tiles = [None] * SPC
            ex_tiles = [None] * SPC

            def emit_scores(h, qh, kh):
                for seg in range(SPC):
                    scT = pa_p.tile([128, 2, L], F32, tag="pa", bufs=4,
                                    name="scT")
                    for lkc in range(2):
                        nc.tensor.matmul(
                            scT[:, lkc, :],
                            kh[:, seg * L + lkc * 128: seg * L + (lkc + 1) * 128],
                            qh[:, seg * L:(seg + 1) * L],
                        )
                    e_t = ex_p.tile([128, 2, L], BF, tag="ex")
                    nc.scalar.activation(
                        out=e_t[:],
                        in_=scT[:],
                        func=mybir.ActivationFunctionType.Exp,
                        scale=SCALE,
                    )
                    sc_tiles[seg] = scT
                    ex_tiles[seg] = e_t

            def attention_tail(h):
                # All 8 sums matmuls first under a single ones-LDWEIGHTS
                # (an LDW that follows a stop-matmul is not dual-issued and
                # costs ~95ns of PE; batching drops 3 of the 4 per head),
                # then the AV pairs.  The reciprocal/broadcast for seg s run
                # on DVE/GpSimd while the PE is still on AV matmuls, so the
                # normalize mul follows AV(s) with no serial recip chain.
                invBs = [None] * SPC
                for seg in range(SPC):
                    scT = sc_tiles[seg]
                    e_t = ex_tiles[seg]
                    for lkc in range(2):
                        nc.tensor.matmul(
                            scT[0:1, 1, :],
                            ones[:],
                            e_t[:, lkc, :],
                            start=(lkc == 0),
                            stop=(lkc == 1),
                        )
                for seg in range(SPC):
                    inv = st_p.tile([1, L], F32, tag="st", bufs=4)
                    nc.vector.reciprocal_approx_fast(
                        out=inv[:], in_=sc_tiles[seg][0:1, 1, :])
                    invB = ex_p.tile([128, L], F32, tag="invb")
                    nc.gpsimd.partition_broadcast(invB[:], inv[:])
                    invBs[seg] = invB
                for seg in range(SPC):
                    scT = sc_tiles[seg]
                    e_t = ex_tiles[seg]
                    sc_tiles[seg] = None
                    ex_tiles[seg] = None
                    for lkc in range(2):
                        nc.tensor.matmul(
                            scT[:, 0, :],
                            vtok_sb[:, h, seg * 2 + lkc, :],
                            e_t[:, lkc, :],
                            start=(lkc == 0),
                            stop=(lkc == 1),
                        )
                    nc.vector.tensor_mul(aT_sb[:, seg, h, :], scT[:, 0, :],
                                         invBs[seg][:])

            prev_h = None
            for h in range(H):
                qh = qk_p.tile([128, TOK], BF, tag="qk")
                kh = qk_p.tile([128, TOK], BF, tag="qk")
                proj_chunk(h, qh)
                proj_chunk(16 + h, kh)
                if prev_h is not None:
                    attention_tail(prev_h)
                emit_scores(h, qh, kh)
                prev_h = h
            attention_tail(H - 1)

            # ---- output projection (token-major) ----
            # Wout is streamed in four 2MB e-quarters (one linear DMA each)
            # instead of held resident; the freed 32KB/partition goes to
            # deeper W-chunk prefetch.  LDWEIGHTS (one per aT tile per
            # quarter) hides under the previous matmul's streaming.
            for eq in range(4):
                wq_t = w_p.tile([128, DT, 512], BF, tag="wo", bufs=2,
                                name="wq_t")
                nc.sync.dma_start(out=wq_t[:], in_=wout_d[eq])
                for lc in range(TOK // 128):
                    seg, lqc = lc // 2, lc % 2
                    po = pp_p.tile([128, 512], F32, tag="pp", name="po")
                    for dt in range(DT):
                        nc.tensor.matmul(
                            po[:],
                            aT_sb[:, seg, dt, ts(lqc, 128)],
                            wq_t[:, dt, :],
                            start=(dt == 0),
                            stop=(dt == DT - 1),
                        )
                    ob = ou_p.tile([128, 512], F32, tag="ou")
                    nc.vector.tensor_copy(out=ob[:], in_=po[:])
                    nc.sync.dma_start(
                        out=out_d[lc * 128:(lc + 1) * 128,
                                  eq * 512:(eq + 1) * 512],
                        in_=ob[:],
                    )

    nc.compile()
    _dedupe_ldweights(nc)
    return nc


def _dedupe_ldweights(nc):
    """Drop InstLdweights whose weights are already resident in the PE array.

    tile_legalize emits one LDWEIGHTS per matmul; consecutive matmuls that
    share the stationary operand (projection token-halves, out-proj eq
    pairs) reload identical weights, costing ~97ns of PE pipe each.  Walk
    each block's PE stream tracking the loaded-weights key and delete
    reloads.  Only semaphore-free LDWEIGHTS are dropped, so the sync graph
    is untouched; EVENT_SEMAPHORE/DRAIN between pairs don't disturb the
    array, any other PE instruction conservatively invalidates the key.
    """
    from concourse import mybir

    PE = mybir.EngineType.PE
    dropped = 0
    for f in nc.m.functions:
        for blk in f.blocks:
            insts = blk.instructions
            loaded = None
            to_drop = []
            for idx, x in enumerate(insts):
                if getattr(x, "engine", None) != PE:
                    continue
                nm = type(x).__name__
                if nm == "InstLdweights":
                    si = x.sync_info
                    clean = si is None or (not si.on_wait and not si.on_update)
                    key = (str(x.ins[0]), str(x.is_transpose),
                           str(x.perf_mode), str(x.tile_position))
                    if clean and loaded == key:
                        to_drop.append(idx)
                    else:
                        loaded = key
                elif nm == "InstMatmult":
                    continue
                elif nm in ("InstEventSemaphore", "InstDrain"):
                    continue
                else:
                    loaded = None
            for idx in reversed(to_drop):
                del insts[idx]
            blk.instructions = insts
            dropped += len(to_drop)
    return dropped


def get_program():
    global _PROGRAM
    if _PROGRAM is None:
        _PROGRAM = _build_program()
    return _PROGRAM


def make_in_maps(x, Wqkv, b_qkv):
    """Host-side shard + layout prep (bf16 casts, transposes, tiling)."""
    bf16 = ml_dtypes.bfloat16
    x = np.asarray(x, dtype=np.float32)
    Wqkv = np.asarray(Wqkv, dtype=np.float32)
    b_qkv = np.asarray(b_qkv, dtype=np.float32)

    xs = x.reshape(B, NSEG, SEGMENT, D)[:, :, ::DIL, :]     # [2,16,256,2048]
    xs_flat = xs.reshape(PAIRS, L, D)

    # lhsT tiles packed partition-major: wt[c, p, dt*128+j] = WqkvT[dt*128+p,
    # c*128+j] so one chunk is a single linear per-partition DMA.
    wt = np.ascontiguousarray(
        Wqkv.reshape(NCHUNK, 128, DT, 128).transpose(0, 3, 2, 1)
        .reshape(NCHUNK, 128, DT * 128)
    ).astype(bf16)                                          # [48,128,2048]
    bqt = np.ascontiguousarray(b_qkv.reshape(NCHUNK, 128).T)  # [128,48] f32

    in_maps = []
    for i in range(N_CORES):
        tok = xs_flat[SPC * i:SPC * (i + 1)].reshape(TOK, D)
        xst = np.ascontiguousarray(
            tok.T.reshape(DT, 128, TOK).transpose(1, 0, 2)
            .reshape(128, DT * TOK)).astype(bf16)
        in_maps.append({"xst": xst, "wqkv_t": wt, "bq_t": bqt})
    return in_maps


def make_wout_tiled(Wout):
    Wout = np.asarray(Wout, dtype=np.float32)
    # [eq, p, dt*512+j] = Wout[eq*512+j, dt*128+p]: one linear DMA/quarter
    return np.ascontiguousarray(
        Wout.T.reshape(DT, 128, 4, 512).transpose(2, 1, 0, 3)
        .reshape(4, 128, DT * 512)).astype(ml_dtypes.bfloat16)


def kernel(x, Wqkv, b_qkv, Wout, b_out):
    from concourse import bass_utils

    nc = get_program()
    in_maps = make_in_maps(x, Wqkv, b_qkv)
    wot = make_wout_tiled(Wout)
    for m in in_maps:
        m["wout_t"] = wot

    res = bass_utils.run_bass_kernel_spmd(
        nc, in_maps, core_ids=list(range(N_CORES)))
    outs = [res.results[i]["out"] for i in range(N_CORES)]
    full = np.concatenate(outs, axis=0) + np.asarray(b_out, dtype=np.float32)
    return np.ascontiguousarray(full.reshape(B, NSEG * L, D), dtype=np.float32)



# revision 13
# speedup vs baseline: 1.0060x; 1.0060x over previous
"""Dilated segment attention on 8 TRN2 NeuronCores (Bass/Tile).

Problem (hardcoded from spec):
  x [2, 8192, 2048] f32, Wqkv [6144, 2048], b_qkv [6144], Wout [2048, 2048],
  b_out [2048].  segment=512, dilation=2 -> 16 segments of L=256 dilated
  tokens per batch; per-segment 16-head attention (hd=128); fused qkv and
  out projections.  Output [2, 4096, 2048] f32.

Sharding: the 32 (batch, segment) instances are independent -> 4 per core.
Host pre-gathers the dilated tokens, pre-transposes/pre-tiles operands and
casts to bf16 (compute precision; measured end-to-end rel err ~5e-3).

Per-core dataflow (all matmuls K=128, bf16):
  qkv proj   : feature-major  qkvT[e, tok] = W-tile.T @ xsT-tile  (accum 16
               d-tiles); psum->sbuf drain + b_qkv on DVE (ScalarE is kept
               free for the softmax exps)
  scores     : scoresT[lk, lq] = kT.T @ qT        (per seg, head)
  softmax    : exp on ScalarE (scale=1/sqrt(hd); scores are provably in
               [-6, 6] so no max subtraction), pipelined two projection
               chunks ahead of the sums/AV consumers; sums via ones-matmul,
               normalize on DVE
  AV         : outT[hd, lq] = v[lk, hd].T @ expT[lk, lq]  (into the scores
               psum bank, post-exp)
  out proj   : out[l, e] = aT-tile.T @ WoutT-tile  (accum 16 head-tiles,
               token-major, so the HBM store is linear)
b_out is applied on the host (purely linear post-op); b_qkv is applied
on-chip (DVE tensor_scalar bias) since it feeds the softmax nonlinearity.
"""

import numpy as np
import ml_dtypes

B = 2
S = 8192
D = 2048
H = 16
HD = 128
SEGMENT = 512
DIL = 2
NSEG = S // SEGMENT          # 16
L = SEGMENT // DIL           # 256 dilated tokens per segment
N_CORES = 8
PAIRS = B * NSEG             # 32 independent (b, n) instances
SPC = PAIRS // N_CORES       # 4 segments per core
TOK = SPC * L                # 1024 tokens per core
DT = D // 128                # 16 contraction tiles
NCHUNK = 3 * D // 128        # 48 qkv feature chunks (16 q, 16 k, 16 v)
SCALE = 1.0 / float(np.sqrt(HD))

_PROGRAM = None


def _build_program():
    import concourse.bass as bass
    import concourse.bacc as bacc
    import concourse.tile as tile
    from concourse import mybir

    BF = mybir.dt.bfloat16
    F32 = mybir.dt.float32
    ts = bass.ts

    nc = bacc.Bacc("TRN2", target_bir_lowering=False, debug=False,
                   num_devices=N_CORES)

    xst_d = nc.dram_tensor("xst", [128, DT * TOK], BF, kind="ExternalInput")
    wqkv_d = nc.dram_tensor("wqkv_t", [NCHUNK, 128, DT * 128], BF,
                            kind="ExternalInput")
    wout_d = nc.dram_tensor("wout_t", [4, 128, DT * 512], BF, kind="ExternalInput")
    bq_d = nc.dram_tensor("bq_t", [128, NCHUNK], F32, kind="ExternalInput")
    out_d = nc.dram_tensor("out", [TOK, D], F32, kind="ExternalOutput")

    with tile.TileContext(nc) as tc:
        with (
            tc.tile_pool(name="const", bufs=1) as const_p,
            tc.tile_pool(name="big", bufs=1) as big_p,
            tc.tile_pool(name="wq", bufs=8) as w_p,
            tc.tile_pool(name="qk", bufs=4) as qk_p,
            tc.tile_pool(name="vt", bufs=2) as vt_p,
            tc.tile_pool(name="ex", bufs=4) as ex_p,
            tc.tile_pool(name="st", bufs=2) as st_p,
            tc.tile_pool(name="ou", bufs=2) as ou_p,
            tc.tile_pool(name="pp", bufs=4, space="PSUM") as pp_p,
            tc.tile_pool(name="pa", bufs=2, space="PSUM") as pa_p,
        ):
            bq_sb = const_p.tile([128, NCHUNK], F32)
            nc.sync.dma_start(out=bq_sb[:], in_=bq_d[:])
            ones = const_p.tile([128, 1], BF)
            nc.gpsimd.memset(ones[:], 1.0)

            # One linear 512KB DMA per weight chunk, 4 x 1MB for xst, and 4
            # prefetched W chunks: transfers run concurrently at the HBM
            # aggregate rate, so everything lands ~6-7us after the last
            # dispatch; fewer dispatches (~650ns each on Sync) land it
            # sooner.
            first_w = w_p.tile([128, DT * 128], BF, tag="w", name="first_w")
            nc.sync.dma_start(out=first_w[:], in_=wqkv_d[32])
            xst_sb = big_p.tile([128, DT, TOK], BF)
            pre_w = {0: first_w}
            for kk in range(4):
                nc.sync.dma_start(
                    out=xst_sb[:, 4 * kk:4 * (kk + 1), :],
                    in_=xst_d[:, 4 * kk * TOK:4 * (kk + 1) * TOK],
                )
                if kk < 3:
                    wpre = w_p.tile([128, DT * 128], BF, tag="w",
                                    name=f"w{33 + kk}")
                    nc.sync.dma_start(out=wpre[:], in_=wqkv_d[33 + kk])
                    pre_w[1 + kk] = wpre

            # Warm-up matmuls: the PE DVFS ramp takes ~3.5us of continuous
            # activity to reach 2.4GHz (first ~8 real matmuls measured at
            # 427ns instead of 216ns).  Burn ~5us of full-width matmuls on
            # memset garbage while the initial DMA wave (~15us incl. the
            # 7us engine preamble) is in flight, so the clock is at full
            # speed when real data lands.
            scratch = const_p.tile([128, 512], BF)
            nc.gpsimd.memset(scratch[:], 0.0)
            warm_ps = pp_p.tile([128, 512], F32, tag="pp", name="warm_ps")
            for _ in range(18):
                nc.tensor.matmul(warm_ps[:], scratch[:, 0:128], scratch[:],
                                 start=True, stop=True)
            vtok_sb = big_p.tile([128, H, SPC * 2, 128], BF)
            aT_sb = big_p.tile([128, SPC, H, L], BF)

            def proj_chunk(c, out_tile, wck=None):
                """qkvT chunk c: out_tile[128, TOK] bf16 = (Wqkv chunk).T @ xsT + b.

                The psum->sbuf drain (+bias) runs on the DVE, not ScalarE:
                ScalarE is reserved for the softmax exps so they never queue
                behind a 2x700ns drain pair (measured ~165-240ns of PE stall
                per segment waiting on e_t with the drains on ScalarE).
                """
                if wck is None:
                    wck = w_p.tile([128, DT * 128], BF, tag="w")
                    nc.sync.dma_start(out=wck[:], in_=wqkv_d[c])
                pss = [pp_p.tile([128, 512], F32, tag="pp", name=f"ps{half}")
                       for half in range(2)]
                for dt in range(DT):
                    for half in range(2):
                        nc.tensor.matmul(
                            pss[half][:],
                            wck[:, ts(dt, 128)],
                            xst_sb[:, dt, ts(half, 512)],
                            start=(dt == 0),
                            stop=(dt == DT - 1),
                        )
                for half in range(2):
                    nc.vector.tensor_scalar_add(
                        out_tile[:, ts(half, 512)],
                        pss[half][:],
                        bq_sb[:, c:c + 1],
                    )

            # ---- v projection (feature-major) + transpose to token-major ----
            # One transposing DMA per head (xbar transpose, ~261GB/s): row
            # tok = tc*128+p of vt.T lands at vtok[p, tc, :], exactly the AV
            # stationary layout.  Keeps ~28us of transposes off the PE.
            # Emitted one chunk behind the projection so the DMA never waits
            # on the ScalarE psum->sbuf drain in the static stream.
            def v_transposes(h, vt_tile):
                nc.sync.dma_start(out=vtok_sb[:, h, :, :], in_=vt_tile[:],
                                  transpose=True)

            prev_v = None
            for h in range(H):
                vt_tile = vt_p.tile([128, TOK], BF, tag="vt")
                proj_chunk(32 + h, vt_tile, wck=pre_w.get(h))
                if prev_v is not None:
                    v_transposes(h - 1, prev_v)
                prev_v = vt_tile
            v_transposes(H - 1, prev_v)

            # ---- per-head: q/k projection then attention over 4 segments ----
            # scoresT[lk, lq] directly (operands swapped): exp is
            # layout-agnostic (scores provably small -> no max pass),
            # softmax sums go along partitions via a ones-matmul, expT
            # feeds AV untransposed, and the normalization happens at the
            # psum->sbuf copy of the AV output.
            #
            # Two-stage software pipeline against the projections:
            #   iter h: projq(h), projk(h),
            #           sums/AV/normalize(h-1)   [frees scT(h-1) banks]
            #           scores(h) + exps(h)      [exps run on an otherwise
            #                                     idle ScalarE under head
            #                                     h+1's ~13.6us of proj MMs]
            # so the sums/AV never wait on exp latency (measured ~360ns/seg
            # when exp was consumed within the same attention block).  The
            # sums and AV reuse the scores psum bank (sums -> [0:1,1,:],
            # AV -> [:,0,:], both emitted after the exp consumed the
            # scores), so attention fits in 4 banks alongside the 4
            # projection banks.  scores(h) sits ~1.7us of sums/AV work
            # after projk(h)'s last matmul, covering the DVE drain latency
            # of kh.

            sc_tiles = [None] * SPC
            ex_tiles = [None] * SPC

            def emit_scores(h, qh, kh):
                for seg in range(SPC):
                    scT = pa_p.tile([128, 2, L], F32, tag="pa", bufs=4,
                                    name="scT")
                    for lkc in range(2):
                        nc.tensor.matmul(
                            scT[:, lkc, :],
                            kh[:, seg * L + lkc * 128: seg * L + (lkc + 1) * 128],
                            qh[:, seg * L:(seg + 1) * L],
                        )
                    e_t = ex_p.tile([128, 2, L], BF, tag="ex")
                    nc.scalar.activation(
                        out=e_t[:],
                        in_=scT[:],
                        func=mybir.ActivationFunctionType.Exp,
                        scale=SCALE,
                    )
                    sc_tiles[seg] = scT
                    ex_tiles[seg] = e_t

            def attention_tail(h):
                # All 8 sums matmuls first under a single ones-LDWEIGHTS
                # (an LDW that follows a stop-matmul is not dual-issued and
                # costs ~95ns of PE; batching drops 3 of the 4 per head),
                # then the AV pairs.  The reciprocal/broadcast for seg s run
                # on DVE/GpSimd while the PE is still on AV matmuls, so the
                # normalize mul follows AV(s) with no serial recip chain.
                invBs = [None] * SPC
                for seg in range(SPC):
                    scT = sc_tiles[seg]
                    e_t = ex_tiles[seg]
                    for lkc in range(2):
                        nc.tensor.matmul(
                            scT[0:1, 1, :],
                            ones[:],
                            e_t[:, lkc, :],
                            start=(lkc == 0),
                            stop=(lkc == 1),
                        )
                for seg in range(SPC):
                    inv = st_p.tile([1, L], F32, tag="st", bufs=4)
                    nc.vector.reciprocal_approx_fast(
                        out=inv[:], in_=sc_tiles[seg][0:1, 1, :])
                    invB = ex_p.tile([128, L], F32, tag="invb")
                    nc.gpsimd.partition_broadcast(invB[:], inv[:])
                    invBs[seg] = invB
                for seg in range(SPC):
                    scT = sc_tiles[seg]
                    e_t = ex_tiles[seg]
                    sc_tiles[seg] = None
                    ex_tiles[seg] = None
                    for lkc in range(2):
                        nc.tensor.matmul(
                            scT[:, 0, :],
                            vtok_sb[:, h, seg * 2 + lkc, :],
                            e_t[:, lkc, :],
                            start=(lkc == 0),
                            stop=(lkc == 1),
                        )
                    nc.vector.tensor_mul(aT_sb[:, seg, h, :], scT[:, 0, :],
                                         invBs[seg][:])

            prev_h = None
            for h in range(H):
                qh = qk_p.tile([128, TOK], BF, tag="qk")
                kh = qk_p.tile([128, TOK], BF, tag="qk")
                proj_chunk(h, qh)
                proj_chunk(16 + h, kh)
                if prev_h is not None:
                    attention_tail(prev_h)
                emit_scores(h, qh, kh)
                prev_h = h
            attention_tail(H - 1)

            # ---- output projection (token-major) ----
            # Wout is streamed in four 2MB e-quarters (one linear DMA each)
            # instead of held resident; the freed 32KB/partition goes to
            # deeper W-chunk prefetch.  LDWEIGHTS (one per aT tile per
            # quarter) hides under the previous matmul's streaming.
            for eq in range(4):
                wq_t = w_p.tile([128, DT, 512], BF, tag="wo", bufs=2,
                                name="wq_t")
                nc.sync.dma_start(out=wq_t[:], in_=wout_d[eq])
                for lc in range(TOK // 128):
                    seg, lqc = lc // 2, lc % 2
                    po = pp_p.tile([128, 512], F32, tag="pp", name="po")
                    for dt in range(DT):
                        nc.tensor.matmul(
                            po[:],
                            aT_sb[:, seg, dt, ts(lqc, 128)],
                            wq_t[:, dt, :],
                            start=(dt == 0),
                            stop=(dt == DT - 1),
                        )
                    ob = ou_p.tile([128, 512], F32, tag="ou")
                    nc.vector.tensor_copy(out=ob[:], in_=po[:])
                    nc.sync.dma_start(
                        out=out_d[lc * 128:(lc + 1) * 128,
                                  eq * 512:(eq + 1) * 512],
                        in_=ob[:],
                    )

    nc.compile()
    _dedupe_ldweights(nc)
    return nc


def _dedupe_ldweights(nc):
    """Drop InstLdweights whose weights are already resident in the PE array.

    tile_legalize emits one LDWEIGHTS per matmul; consecutive matmuls that
    share the stationary operand (projection token-halves, out-proj eq
    pairs) reload identical weights, costing ~97ns of PE pipe each.  Walk
    each block's PE stream tracking the loaded-weights key and delete
    reloads.  Only semaphore-free LDWEIGHTS are dropped, so the sync graph
    is untouched; EVENT_SEMAPHORE/DRAIN between pairs don't disturb the
    array, any other PE instruction conservatively invalidates the key.
    """
    from concourse import mybir

    PE = mybir.EngineType.PE
    dropped = 0
    for f in nc.m.functions:
        for blk in f.blocks:
            insts = blk.instructions
            loaded = None
            to_drop = []
            for idx, x in enumerate(insts):
                if getattr(x, "engine", None) != PE:
                    continue
                nm = type(x).__name__
                if nm == "InstLdweights":
                    si = x.sync_info
                    clean = si is None or (not si.on_wait and not si.on_update)
                    key = (str(x.ins[0]), str(x.is_transpose),
                           str(x.perf_mode), str(x.tile_position))
                    if clean and loaded == key:
                        to_drop.append(idx)
                    else:
                        loaded = key
                elif nm == "InstMatmult":
                    continue
                elif nm in ("InstEventSemaphore", "InstDrain"):
                    continue
                else:
                    loaded = None
            for idx in reversed(to_drop):
                del insts[idx]
            blk.instructions = insts
            dropped += len(to_drop)
    return dropped


def get_program():
    global _PROGRAM
    if _PROGRAM is None:
        _PROGRAM = _build_program()
    return _PROGRAM


def make_in_maps(x, Wqkv, b_qkv):
    """Host-side shard + layout prep (bf16 casts, transposes, tiling)."""
    bf16 = ml_dtypes.bfloat16
    x = np.asarray(x, dtype=np.float32)
    Wqkv = np.asarray(Wqkv, dtype=np.float32)
    b_qkv = np.asarray(b_qkv, dtype=np.float32)

    xs = x.reshape(B, NSEG, SEGMENT, D)[:, :, ::DIL, :]     # [2,16,256,2048]
    xs_flat = xs.reshape(PAIRS, L, D)

    # lhsT tiles packed partition-major: wt[c, p, dt*128+j] = WqkvT[dt*128+p,
    # c*128+j] so one chunk is a single linear per-partition DMA.
    wt = np.ascontiguousarray(
        Wqkv.reshape(NCHUNK, 128, DT, 128).transpose(0, 3, 2, 1)
        .reshape(NCHUNK, 128, DT * 128)
    ).astype(bf16)                                          # [48,128,2048]
    bqt = np.ascontiguousarray(b_qkv.reshape(NCHUNK, 128).T)  # [128,48] f32

    in_maps = []
    for i in range(N_CORES):
        tok = xs_flat[SPC * i:SPC * (i + 1)].reshape(TOK, D)
        xst = np.ascontiguousarray(
            tok.T.reshape(DT, 128, TOK).transpose(1, 0, 2)
            .reshape(128, DT * TOK)).astype(bf16)
        in_maps.append({"xst": xst, "wqkv_t": wt, "bq_t": bqt})
    return in_maps


def make_wout_tiled(Wout):
    Wout = np.asarray(Wout, dtype=np.float32)
    # [eq, p, dt*512+j] = Wout[eq*512+j, dt*128+p]: one linear DMA/quarter
    return np.ascontiguousarray(
        Wout.T.reshape(DT, 128, 4, 512).transpose(2, 1, 0, 3)
        .reshape(4, 128, DT * 512)).astype(ml_dtypes.bfloat16)


def kernel(x, Wqkv, b_qkv, Wout, b_out):
    from concourse import bass_utils

    nc = get_program()
    in_maps = make_in_maps(x, Wqkv, b_qkv)
    wot = make_wout_tiled(Wout)
    for m in in_maps:
        m["wout_t"] = wot

    res = bass_utils.run_bass_kernel_spmd(
        nc, in_maps, core_ids=list(range(N_CORES)))
    outs = [res.results[i]["out"] for i in range(N_CORES)]
    full = np.concatenate(outs, axis=0) + np.asarray(b_out, dtype=np.float32)
    return np.ascontiguousarray(full.reshape(B, NSEG * L, D), dtype=np.float32)

